# revision 74
# baseline (speedup 1.0000x reference)
"""Causal self-attention (B=4, T=2048, H=1024, NH=16, HD=64) on 8 trn2 cores.

Sharding: tensor-parallel over heads - core c computes heads 2c, 2c+1 for all
batches; Q/K/V weights column-sharded by head. All matmul inputs are bf16
(full PE rate at any output width in the cost model; rel err ~5e-3 << 2e-2).

Per-core dataflow:
  projection (per batch b, X^T bf16 k-tiles held in one [128, 8, T] tile):
    Q^T, K^T [128 d2, T] via W-stationary matmuls; bq folded into Q via the
    DVE PSUM->SBUF move (bk dropped: a key-independent additive term
    cancels in softmax; bv added on host post-normalize).
    V in NATURAL [t, d] layout via X-stationary matmuls (out [128 t, 128 d])
    so no PE transpose is needed; DVE extracts V into per-t-tile va tiles
    [128, 2*(64+1)] bf16 with a ones column per head (PV emits the softmax
    denominator as output column 64).
  attention (per b, query block ib of 512, key tile jt):
    S matmuls are plain start=True (no PSUM mask preload); the causal mask
    is applied AFTER exp: P^T = exp(S^T/8) -> SBUF bf16, then one DVE
    tensor_mul zeroes the strictly-lower triangle of the 128-col diagonal
    block (bf16 SBUF operands -> DVE 2x mode; replaces the much larger
    f32 SBUF->PSUM mask copies of the earlier design).
    PV with P^T [128 j, 128 i] as STATIONARY and va [128 j, 65] moving:
    O[i, d]/den accumulate in PSUM [128 i, 4*65] per (ib, h) across jt.
  output: O [B, 2, T, 65] f32 (col 64 = denominator); host divides and
  adds bv - output is already [b, h, t, d] ordered.
  Schedule (TimelineSim 190.4us vs 225.2us for the mask-preload design):
  - projection chunk units are deferred to just before their deadline and
    spread into the attention jt loops so the ACT-bound attention
    stretches (exp is ~36.7us/batch vs ~21.9us of attention PE work)
    always have PE work to interleave;
  - cross-block S warmup: each block's PSUM setup + first two S tiles are
    pre-emitted at slot njt-2 of the previous block, bridging the ~2us
    Activation-engine drain during the previous block's PV-only tail;
  - x DMAs are n-major quarters in kk-halves (the serial DMA resource
    runs in issue order; fine grain avoids head-of-line blocking), with
    weights ordered wqt, x-quarter-0, wkt, wvt so batch 0's first
    projection starts ~5us in at full PE clock (dummy-matmul p-state
    warmup covers the ramp);
  - all DMAs go to HWDGE queues (sync, plus scalar only where exp is
    quiescent); per-DMA completion semaphores cost ~900ns, so the final
    i-tile drains leave in pairs with the last pair as small as possible.
"""

import ml_dtypes
import numpy as np

B, T, H, NH = 4, 2048, 1024, 16
HD = H // NH  # 64
NCORES = 8
HPC = NH // NCORES  # heads per core = 2
BT = B * T
E = HD + 1  # 65: head dim + denominator column

_CACHE = {}


def _build(reps=1):
    import contextlib
    from contextlib import ExitStack

    import concourse.mybir as mybir
    import concourse.tile as tile
    from concourse import bacc

    F32 = mybir.dt.float32
    BF16 = mybir.dt.bfloat16

    nc = bacc.Bacc("TRN2", target_bir_lowering=False, num_devices=NCORES)

    xt = nc.declare_dram_parameter("xt", [H, BT], BF16, isOutput=False)
    wqt = nc.declare_dram_parameter("wqt", [H, 128], BF16, isOutput=False)
    wkt = nc.declare_dram_parameter("wkt", [H, 128], BF16, isOutput=False)
    wvt = nc.declare_dram_parameter("wvt", [H, 128], BF16, isOutput=False)
    bq = nc.declare_dram_parameter("bq", [128, 1], F32, isOutput=False)
    out = nc.declare_dram_parameter("out", [B, HPC, T, E], F32, isOutput=True)

    # 0/1 causal keep-mask for one 128x128 diagonal tile, duplicated for the
    # two heads: tri2[j, h*128 + i] = 0 where key j > query i (strict lower
    # triangle in (j, i)), else 1.
    jj = np.arange(128)[:, None]
    ii = np.arange(128)[None, :]
    tri = np.where(jj > ii, 0.0, 1.0).astype(ml_dtypes.bfloat16)
    tri2_dram = nc.inline_tensor(np.concatenate([tri, tri], axis=1), name="tri2")

    NKT = H // 128  # 8 contraction tiles
    NIB = T // 512  # 4 query blocks
    NJT = T // 128  # 16 key tiles

    with tile.TileContext(nc) as tc:
        with ExitStack() as ctx:
            const = ctx.enter_context(tc.tile_pool(name="const", bufs=1))
            xpool = ctx.enter_context(tc.tile_pool(name="xpool", bufs=2))
            qkpool = ctx.enter_context(tc.tile_pool(name="qkpool", bufs=2))
            vapool = ctx.enter_context(tc.tile_pool(name="vapool", bufs=2))
            # pt tiles for a whole ib (up to 16 key tiles) stay live until the
            # diagonal PV groups consume them, plus lookahead
            ppool = ctx.enter_context(tc.tile_pool(name="ppool", bufs=20))
            opool = ctx.enter_context(tc.tile_pool(name="opool", bufs=3))
            # PSUM: S wide [128,1024] x2 (4 banks) + proj [128,512] x2 (2
            # banks) + two O accumulators [128, 4*65] x1 (2 banks) = 8 banks.
            psS = ctx.enter_context(tc.tile_pool(name="psS", bufs=2, space="PSUM"))
            psP = ctx.enter_context(tc.tile_pool(name="psP", bufs=2, space="PSUM"))
            psO = ctx.enter_context(tc.tile_pool(name="psO", bufs=1, space="PSUM"))

            # --- constants / weights. The serial DMA resource executes
            # transfers roughly in issue order, so the startup-critical loads
            # (wqt, then batch 0's x quarters, emitted by the scheduler right
            # after this block) must issue before the non-critical ones. ---
            wt_sb = const.tile([128, 3 * H], BF16)
            bq_sb = const.tile([128, 1], F32)
            tri2_sb = const.tile([128, HPC * 128], BF16)
            tri2_v = tri2_sb.rearrange("p (h i) -> p h i", h=HPC)

            def load_weight(p, w):
                # on sync: the serial DMA resource runs transfers in issue
                # order, and these must land between d0 and d1. kk-halves:
                # the dependent projection chain starts on the first half.
                for hk in range(2):
                    nc.sync.dma_start(
                        wt_sb[
                            :, p * H + hk * 512 : p * H + (hk + 1) * 512
                        ].rearrange("p (kt c) -> p kt c", kt=NKT // 2),
                        w.rearrange("(kt p) c -> p kt c", p=128)[
                            :, hk * 4 : (hk + 1) * 4, :
                        ],
                    )

            nc.sync.dma_start(
                wt_sb[:, 0:H].rearrange("p (kt c) -> p kt c", kt=NKT),
                wqt.rearrange("(kt p) c -> p kt c", p=128),
            )

            def load_late_consts():
                # bq is tiny (56ns) but gates q0's PSUM->SBUF copy, which
                # gates the psP ring for v0: load it right after d0, before
                # the other weights
                nc.sync.dma_start(bq_sb[:], bq[:])
                load_weight(1, wkt)
                load_weight(2, wvt)
                nc.scalar.dma_start(tri2_sb[:], tri2_dram[:])

            Exp = mybir.ActivationFunctionType.Exp
            # preload the Exp activation table while DMAs are in flight
            warm = const.tile([128, 1], BF16, name="warm")
            nc.scalar.activation(warm[:], wt_sb[:, 0:1], Exp, bias=0.0, scale=1.0)

            # PE p-state warmup: keep the PE continuously busy with dummy
            # matmuls until the first projection's inputs land, so the
            # ~3us frequency ramp (0.65 -> 1.2 -> 2.4 GHz) is paid on idle
            # time instead of on the first real matmuls.
            zz = const.tile([128, 512], BF16, name="zz")
            nc.gpsimd.memset(zz[:], 0.0)
            pwarm = psP.tile([128, 512], F32, name="pwarm", tag="pw")
            for _ in range(10):
                nc.tensor.matmul(pwarm[:], zz[:, 0:128], zz[:], start=True, stop=True)

            state = {}  # per-batch x/q/k views + per-(b, jt) va tiles

            def proj_units(b):
                units = {}

                def alloc_x(b=b):
                    x_sb = xpool.tile([128, NKT * T], BF16, name="x_sb", tag="x")
                    state[b] = {"xv": x_sb.rearrange("p (kt c) -> p kt c", kt=NKT)}

                def dma_unit(n, b=b):
                    # one n-major quarter: cols [n*512, (n+1)*512) of every
                    # contraction tile, in a single strided DMA. Batch 0's
                    # first quarter is split into kk halves so the first
                    # projection matmuls can start after ~half the transfer.
                    if n == 0:
                        alloc_x(b)
                    xv = state[b]["xv"]
                    src = xt.rearrange("(kt p) c -> p kt c", p=128)[
                        :, :, b * T + n * 512 : b * T + (n + 1) * 512
                    ]
                    # kk-halves everywhere: batch 0's first projection can
                    # start on the first half; for later batches the finer
                    # granularity halves head-of-line blocking of the serial
                    # DMA resource (output DMAs free PSUM accumulators)
                    for hk in range(2):
                        ks = slice(hk * 4, (hk + 1) * 4)
                        eng = nc.sync
                        eng.dma_start(
                            xv[:, ks, n * 512 : (n + 1) * 512],
                            src[:, ks, :],
                        )

                def qk_alloc(b=b):
                    st = state[b]
                    if "q" not in st:
                        st["q"] = qkpool.tile([128, T], BF16, name="qt_sb", tag="qt")
                        st["k"] = qkpool.tile([128, T], BF16, name="kt_sb", tag="kt")

                def qk_unit(p, n, q0, q1, b=b):
                    # column range [q0*256, q1*256) of the 512-col chunk;
                    # (0,2) is the whole unit, (0,1)/(1,2) are halves. Each
                    # 256-col span is a complete accumulation group in the
                    # shared bank (sequential groups per bank are legal),
                    # letting the scheduler interleave S emission mid-unit
                    # so the exp backlog doesn't drain during inserts.
                    qk_alloc(b)
                    st = state[b]
                    dest = (st["q"], st["k"])[p]
                    if q0 == 0:
                        st[("ps", p, n)] = psP.tile(
                            [128, 512], F32, name="psp", tag="pw"
                        )
                    ps = st[("ps", p, n)]
                    for kk in range(NKT):
                        nc.tensor.matmul(
                            ps[:, q0 * 256 : q1 * 256],
                            wt_sb[
                                :,
                                (p * NKT + kk) * 128 : (p * NKT + kk + 1) * 128,
                            ],
                            st["xv"][
                                :,
                                kk,
                                n * 512 + q0 * 256 : n * 512 + q1 * 256,
                            ],
                            start=(kk == 0),
                            stop=(kk == NKT - 1),
                        )
                    if q1 == 2:
                        sl = slice(n * 512, (n + 1) * 512)
                        if p == 0:
                            # PSUM->SBUF move folds in the query bias (DVE;
                            # the Pool engine has no PSUM access on TRN2)
                            nc.vector.tensor_scalar_add(
                                dest[:, sl], ps[:], bq_sb[:, 0:1]
                            )
                        else:
                            nc.vector.tensor_copy(dest[:, sl], ps[:])

                def v_unit(grp, c0, c1, b=b):
                    # V in natural [t, d] layout: X^T chunk stationary,
                    # Wv^T chunk moving; out [128 t, 128 d] accumulates over
                    # kk. t-tile range [c0, c1) of the group of 4.
                    st = state[b]
                    if c0 == 0:
                        st[("psv", grp)] = psP.tile(
                            [128, 512], F32, name="psv", tag="pw"
                        )
                    ps = st[("psv", grp)]
                    for c in range(c0, c1):
                        it = grp * 4 + c
                        for kk in range(NKT):
                            nc.tensor.matmul(
                                ps[:, c * 128 : (c + 1) * 128],
                                st["xv"][:, kk, it * 128 : (it + 1) * 128],
                                wt_sb[
                                    :,
                                    (2 * NKT + kk) * 128 : (2 * NKT + kk + 1)
                                    * 128,
                                ],
                                start=(kk == 0),
                                stop=(kk == NKT - 1),
                            )
                        va = vapool.tile(
                            [128, HPC * E], BF16, name=f"va{it}", tag=f"va{it}"
                        )
                        nc.gpsimd.memset(va[:, HD : HPC * E : E], 1.0)
                        dst = va.rearrange("p (h e) -> p h e", h=HPC)[:, :, 0:HD]
                        src = ps[:, c * 128 : (c + 1) * 128].rearrange(
                            "p (h d) -> p h d", h=HPC
                        )
                        nc.vector.tensor_copy(dst, src)
                        state[(b, it)] = va

                for n in range(4):
                    units[f"d{n}"] = lambda n=n: dma_unit(n)
                    units[f"q{n}"] = lambda n=n: qk_unit(0, n, 0, 2)
                    units[f"k{n}"] = lambda n=n: qk_unit(1, n, 0, 2)
                    units[f"v{n}"] = lambda n=n: v_unit(n, 0, 4)
                    units[f"q{n}a"] = lambda n=n: qk_unit(0, n, 0, 1)
                    units[f"q{n}b"] = lambda n=n: qk_unit(0, n, 1, 2)
                    units[f"k{n}a"] = lambda n=n: qk_unit(1, n, 0, 1)
                    units[f"k{n}b"] = lambda n=n: qk_unit(1, n, 1, 2)
                    units[f"v{n}a"] = lambda n=n: v_unit(n, 0, 2)
                    units[f"v{n}b"] = lambda n=n: v_unit(n, 2, 4)
                return units

            def attn_block(b, ib):
                """Return the list of jt-unit callables for attention block
                (b, ib)."""
                njt = 4 * (ib + 1)
                ctx_ib = {}

                def setup_ib(ctx_ib=ctx_ib):
                    ctx_ib["pso"] = [
                        psO.tile(
                            [128, NIB * E], F32, name=f"pso{h}", tag=f"pso{h}"
                        )
                        for h in range(HPC)
                    ]
                    ctx_ib["pts"] = [None] * njt

                def emit_s(jt, ib=ib, ctx_ib=ctx_ib, b=b):
                    st = state[b]
                    v = jt - 4 * ib
                    off = 128 * v if v > 0 else 0
                    nn = 512 - off
                    pss = psS.tile([128, 1024], F32, name="pss", tag="sw")
                    for h in range(HPC):
                        nc.tensor.matmul(
                            pss[:, h * 512 + off : (h + 1) * 512],
                            st["k"][
                                h * HD : (h + 1) * HD,
                                jt * 128 : (jt + 1) * 128,
                            ],
                            st["q"][
                                h * HD : (h + 1) * HD,
                                ib * 512 + off : (ib + 1) * 512,
                            ],
                            start=True,
                            stop=True,
                        )
                    pt = ppool.tile([128, 1024], BF16, name="pt", tag="pt")
                    if v > 0:
                        src = pss.rearrange("p (h i) -> p h i", h=HPC)[
                            :, :, off:512
                        ]
                        dst = pt.rearrange("p (h i) -> p h i", h=HPC)[
                            :, :, off:512
                        ]
                        nc.scalar.activation(dst, src, Exp, bias=0.0, scale=0.125)
                    else:
                        nc.scalar.activation(
                            pt[:], pss[:], Exp, bias=0.0, scale=0.125
                        )
                    if v >= 0:
                        # zero the strictly-lower triangle of the 128-col
                        # diagonal block (both heads, one DVE 2x-mode op)
                        dv = pt.rearrange("p (h i) -> p h i", h=HPC)[
                            :, :, off : off + 128
                        ]
                        nc.vector.tensor_mul(dv, dv, tri2_v)
                    ctx_ib["pts"][jt] = pt

                def warmup(ctx_ib=ctx_ib, njt=njt, setup_ib=setup_ib, emit_s=emit_s):
                    # emitted near the end of the previous block (whose tail
                    # is PV-only): queues this block's first exp so the
                    # Activation engine never drains at block transitions
                    ctx_ib["warm"] = True
                    setup_ib()
                    emit_s(0)
                    if njt > 1:
                        emit_s(1)

                def jt_unit(
                    jt, ib=ib, njt=njt, ctx_ib=ctx_ib, b=b,
                    warmup=warmup, emit_s=emit_s,
                ):
                    if jt == 0:
                        if not ctx_ib.get("warm"):
                            warmup()
                        for w in range(2, min(3, njt)):
                            emit_s(w)
                    if jt + 3 < njt:
                        emit_s(jt + 3)
                    v = jt - 4 * ib
                    if v < 0:
                        return
                    # PSUM start_tensor_calc arms the whole 2KB bank as
                    # pending-zero, so accumulation groups in one bank
                    # must not interleave: emit i-tile v's group (all its
                    # key tiles jt2 <= jt) contiguously at the diagonal.
                    it = v
                    for h in range(HPC):
                        for jt2 in range(jt + 1):
                            pt2 = ctx_ib["pts"][jt2]
                            va2 = state[(b, jt2)]
                            nc.tensor.matmul(
                                ctx_ib["pso"][h][:, it * E : (it + 1) * E],
                                pt2[
                                    :,
                                    h * 512 + it * 128 : h * 512 + (it + 1) * 128,
                                ],
                                va2[:, h * E : (h + 1) * E],
                                start=(jt2 == 0),
                                stop=(jt2 == jt),
                            )
                    if b == B - 1 and ib == NIB - 1:
                        # final ib: drain each i-tile to SBUF as its group
                        # stops; DMA out in i-tile pairs across three queues
                        # so the kernel tail is one copy + one small DMA
                        if "osb_f" not in ctx_ib:
                            ctx_ib["osb_f"] = opool.tile(
                                [128, HPC * NIB * E], F32, name="osbf", tag="osb"
                            )
                        osb = ctx_ib["osb_f"]
                        for h in range(HPC):
                            sl = slice((h * NIB + it) * E, (h * NIB + it + 1) * E)
                            nc.vector.tensor_copy(
                                osb[:, sl],
                                ctx_ib["pso"][h][:, it * E : (it + 1) * E],
                            )
                        if it % 2 == 1:
                            it0 = it - 1
                            # scalar (ACT) queue is safe for the last pair:
                            # all exps are done by then
                            engs = [nc.sync, nc.sync, nc.scalar, nc.sync]
                            for h in range(HPC):
                                sl2 = slice(
                                    (h * NIB + it0) * E, (h * NIB + it + 1) * E
                                )
                                dst = out[
                                    b,
                                    h,
                                    ib * 512 + it0 * 128 : ib * 512
                                    + (it + 1) * 128,
                                    :,
                                ].rearrange("(it p) e -> p it e", it=2)
                                engs[2 * (it0 // 2) + h].dma_start(
                                    dst,
                                    osb[:, sl2].rearrange(
                                        "p (it e) -> p it e", it=2
                                    ),
                                )

                units = [
                    lambda jt=jt: jt_unit(jt) for jt in range(njt)
                ]
                ctx_ib["warmup"] = warmup

                if not (b == B - 1 and ib == NIB - 1):

                    def norm_unit(ib=ib, ctx_ib=ctx_ib, b=b):
                        osb = opool.tile(
                            [128, HPC * NIB * E], F32, name="osb", tag="osb"
                        )
                        for h in range(HPC):
                            nc.vector.tensor_copy(
                                osb[:, h * NIB * E : (h + 1) * NIB * E],
                                ctx_ib["pso"][h][:],
                            )
                        for h in range(HPC):
                            dst = out[b, h, ib * 512 : (ib + 1) * 512, :].rearrange(
                                "(it p) e -> p it e", it=NIB
                            )
                            eng = nc.sync
                            eng.dma_start(
                                dst,
                                osb[:, h * NIB * E : (h + 1) * NIB * E].rearrange(
                                    "p (it e) -> p it e", it=NIB
                                ),
                            )

                    units.append(norm_unit)
                return ctx_ib["warmup"], units

            def emit_schedule():
                P = [proj_units(b) for b in range(B)]

                # batch 0 projections that gate the first attention block.
                # d0 (split in kk halves) issues right after wqt; the
                # non-critical consts (wkt/wvt/bq/tri2) issue next so k0/v0
                # aren't starved, then the remaining quarters.
                P[0]["d0"]()
                load_late_consts()
                for u in ("d1", "d2", "d3", "q0", "k0", "v0"):
                    P[0][u]()

                # proj-unit inserts per attention block: ib -> list of
                # (jt position, batch offset, unit key). Own-batch late
                # chunks fill blocks 0-2; the next batch's dma + chunk-0
                # units fill block 3 (the biggest ACT deficit). k3/v3 are
                # legal inside block 3 up to their first use (jt 6 / jt 9
                # with the 3-tile exp lookahead).
                inserts = {
                    # each chunk unit sits as late as its first use allows
                    # (S(jt) is emitted at jt_unit(jt-3); PV jt uses va jt):
                    # q_n before block n; k_n/v_n may sit a few jt into
                    # block n. This pushes maximum PE work into the late,
                    # ACT-bound stretches - especially batch 3's tail.
                    0: [(1, 0, "q1"), (3, 0, "k1")],
                    1: [(1, 0, "v1"), (3, 0, "q2"), (6, 0, "k2")],
                    2: [
                        (1, 0, "v2"),
                        (6, 0, "q3"),
                        (8, 1, "d0"),
                        (9, 1, "d1"),
                        (10, 1, "q0"),
                    ],
                    3: [
                        (2, 0, "k3"),
                        (5, 0, "v3"),
                        (7, 1, "d2"),
                        (9, 1, "k0"),
                        (11, 1, "v0"),
                        (13, 1, "d3"),
                    ],
                }

                bi = [(b, ib) for b in range(B) for ib in range(NIB)]
                blocks = [attn_block(b, ib) for b, ib in bi]
                for k, (b, ib) in enumerate(bi):
                    njt = 4 * (ib + 1)
                    ins = {}
                    for pos, boff, key in inserts[ib]:
                        if b + boff < B:
                            ins.setdefault(pos, []).append(P[b + boff][key])
                    # pre-warm the NEXT block right after this block's last
                    # S emission (jt_unit njt-4), once the PV-only tail
                    # begins, so exp never starves at the transition
                    if k + 1 < len(blocks):
                        ins.setdefault(njt - 2, []).insert(0, blocks[k + 1][0])
                    for jt, fn in enumerate(blocks[k][1]):
                        fn()
                        for pu in ins.get(jt, ()):
                            pu()

            loop_ctx = tc.For_i(0, reps, 1) if reps > 1 else contextlib.nullcontext()
            with loop_ctx:
                emit_schedule()

    nc.compile()
    return nc


def kernel(hidden_states, attention_mask, Wq, bq, Wk, bk, Wv, bv):
    from concourse.bass_utils import run_bass_kernel_spmd

    if "nc" not in _CACHE:
        _CACHE["nc"] = _build()
    nc = _CACHE["nc"]

    hs = np.asarray(hidden_states, dtype=np.float32)
    Wq, Wk, Wv = (np.asarray(w, dtype=np.float32) for w in (Wq, Wk, Wv))
    bq = np.asarray(bq, dtype=np.float32)
    bv = np.asarray(bv, dtype=np.float32)

    xtb = np.ascontiguousarray(hs.reshape(BT, H).T).astype(ml_dtypes.bfloat16)

    in_maps = []
    for c in range(NCORES):
        sl = slice(c * HPC * HD, (c + 1) * HPC * HD)
        in_maps.append(
            {
                "xt": xtb,
                "wqt": Wq[sl, :].T.astype(ml_dtypes.bfloat16),
                "wkt": Wk[sl, :].T.astype(ml_dtypes.bfloat16),
                "wvt": Wv[sl, :].T.astype(ml_dtypes.bfloat16),
                "bq": np.ascontiguousarray(bq[sl, None]),
            }
        )

    res = run_bass_kernel_spmd(nc, in_maps, core_ids=list(range(NCORES)))

    full = np.empty((B, NH, T, HD), dtype=np.float32)
    for c in range(NCORES):
        o = res.results[c]["out"]  # [B, HPC, T, HD+1]; col HD = denominator
        for h in range(HPC):
            hd = c * HPC + h
            full[:, hd] = o[:, h, :, :HD] / o[:, h, :, HD : HD + 1] + bv[
                hd * HD : (hd + 1) * HD
            ]
    return full
